# revision 21
# baseline (speedup 1.0000x reference)
"""Trainium2 Bass kernel for the per-channel date-conditioning MLP block.

Math (per batch row b, channel c):
    h[c, :]   = gelu(x[b] @ W0[c].T + b0[c])          # 2 -> 32
    out[b, c] = h[c, :] @ W1[c].T + b1[c]             # 32 -> 2

Strategy (per core, batch sharded 8 ways => 2048 rows/core):
  - mm1: out^T layout [c*h, batch]. Inputs are Dekker-split into bf16
    hi/lo (h = Whi@xhi + Whi@xlo + Wlo@xhi, dropped lo*lo ~ 2^-16) and
    fused with the b0 bias via a ones-row => one K=9 bf16 matmul per
    M-tile. Four M-tiles packed concurrently into PE row-groups
    (tile_position=(32j, 0)).
  - ACT: gelu over 2-bank PSUM tiles -> h in SBUF (bias pre-folded so a
    single activation covers two channel-groups).
  - mm2: block-diagonal fp32 lhsT [128, 32] per channel-group (4 channels
    x 32 hidden rows -> 8 outputs, zero-padded to 32 cols); four groups
    packed into PE col-groups (tile_position=(0, 32j)).
  - DVE: + b1 (per-partition scalar) and PSUM -> SBUF drain.
  - One partition-strided DMA per quad compacts the 8 used rows per
    32-row strip to DRAM; host reassembles [batch, 256, 2].
"""

import sys

for _p in ("/opt/trn_rl_repo",):
    if _p not in sys.path:
        sys.path.insert(0, _p)

import ml_dtypes
import numpy as np

B = 16384
C = 256
H = 32
IN_DIM = 2
OUT_DIM = 2
NCORES = 8
BC = B // NCORES  # 2048 batch rows per core
NQ = 16  # "quads": 16 quads x 4 groups x 4 channels = 256 channels
NCHUNK = BC // 512  # batch chunks of 512 (fp32 PSUM-bank matmul max)

BF16 = ml_dtypes.bfloat16

# mm1 input mode: "bf16x2" = Dekker-split bf16 K=9 (fast, ~2e-5 err),
# "fp32" = plain fp32 K=3 (2-pass matmuls, exact).
MM1_MODE = "bf16x2"

_BUILT = {}


def _build():
    import concourse.bass as bass  # noqa: F401
    import concourse.tile as tile
    from concourse import bacc, mybir

    f32 = mybir.dt.float32
    bf16 = mybir.dt.bfloat16
    nc = bacc.Bacc("TRN2", target_bir_lowering=False, debug=False)

    m1dt = bf16 if MM1_MODE == "bf16x2" else f32
    m1k = 9 if MM1_MODE == "bf16x2" else 3
    xt_d = nc.dram_tensor("xt", [m1k, BC], m1dt, kind="ExternalInput").ap()
    w0_d = nc.dram_tensor("w0p", [NQ, 128, 128], m1dt, kind="ExternalInput").ap()
    w1_d = nc.dram_tensor("w1p", [NQ, 128, 128], f32, kind="ExternalInput").ap()
    b1_d = nc.dram_tensor("b1p", [128, NQ], f32, kind="ExternalInput").ap()
    out_d = nc.dram_tensor("out", [NQ, 4, 8, BC], f32, kind="ExternalOutput").ap()

    gelu = mybir.ActivationFunctionType.Gelu

    with tile.TileContext(nc) as tc:
        with (
            tc.tile_pool(name="const", bufs=1) as const,
            tc.tile_pool(name="w0pool", bufs=2) as w0pool,
            tc.tile_pool(name="w1pool", bufs=2) as w1pool,
            tc.tile_pool(name="hpool", bufs=2) as hpool,
            tc.tile_pool(name="opool", bufs=2) as opool,
            tc.tile_pool(name="ps1", bufs=2, space="PSUM") as ps1,
            tc.tile_pool(name="ps2", bufs=2, space="PSUM") as ps2,
        ):
            # First mm1 needs w0[0] + xt group 0 — issue those first so the
            # ACT engine starts as early as possible. b1 isn't needed until
            # the first DVE drain (~25us in).
            w0_first = w0pool.tile([128, 128], m1dt, tag="w0t")
            nc.sync.dma_start(out=w0_first, in_=w0_d[0])
            xt = const.tile([128, BC], m1dt)
            for j in range(4):
                nc.sync.dma_start(out=xt[32 * j : 32 * j + m1k, :], in_=xt_d[:, :])
            b1t = const.tile([128, NQ], f32)
            nc.sync.dma_start(out=b1t, in_=b1_d)

            # Software pipeline: mm2/DVE/store for quad q-1 interleaved
            # chunk-by-chunk with mm1/gelu for quad q, so the PE alternates
            # between feeding the (bottleneck) ACT engine and draining h.
            # h layout per quad: [128, 16, 512] indexed by p = 4*c + j
            # (chunk-major) so consecutive mm1 outputs are contiguous and a
            # single gelu can cover three PSUM banks (N=1536).
            PSPAN = [(0, 3), (3, 3), (6, 3), (9, 3), (12, 3), (15, 1)]
            prev = None  # (q, w1t, hq)
            for qq in range(NQ + 1):
                if qq < NQ:
                    if qq == 0:
                        w0t = w0_first
                    else:
                        w0t = w0pool.tile([128, 128], m1dt, tag="w0t")
                        nc.sync.dma_start(out=w0t, in_=w0_d[qq])
                    w1t = w1pool.tile([128, 128], f32)
                    nc.sync.dma_start(out=w1t, in_=w1_d[qq])
                    hq = hpool.tile([128, 16, 512], f32)
                if prev is not None:
                    ob = opool.tile([128, BC], f32)

                # interleave: mm1/gelu stream for quad qq, mm2 chunks for
                # quad qq-1, round-robin by PSUM-tile group.
                n_steps = max(len(PSPAN), NCHUNK)
                for step in range(n_steps):
                    if qq < NQ and step < len(PSPAN):
                        p0, plen = PSPAN[step]
                        ps = ps1.tile([128, 3, 512], f32, tag="ps")
                        for i in range(plen):
                            p = p0 + i
                            c, j = divmod(p, 4)
                            nc.tensor.matmul(
                                ps[:, i, :],
                                w0t[32 * j : 32 * j + m1k, :],
                                xt[32 * j : 32 * j + m1k, 512 * c : 512 * c + 512],
                                start=True,
                                stop=True,
                                tile_position=(32 * j, 0),
                            )
                        nc.scalar.activation(
                            hq[:, p0 : p0 + plen, :], ps[:, 0:plen, :], gelu
                        )
                    if prev is not None and step < NCHUNK:
                        c = step
                        nsl = slice(512 * c, 512 * c + 512)
                        pq, pw1, phq = prev
                        po = ps2.tile([128, 512], f32)
                        for j in range(4):
                            nc.tensor.matmul(
                                po[32 * j : 32 * j + 32, :],
                                pw1[:, 32 * j : 32 * j + 32],
                                phq[:, 4 * c + j, :],
                                start=True,
                                stop=True,
                                tile_position=(0, 32 * j),
                            )
                        nc.vector.tensor_scalar_add(
                            out=ob[:, nsl], in0=po, scalar1=b1t[:, pq : pq + 1]
                        )
                        if pq == NQ - 1:
                            # tail quad: per-chunk stores overlap the drain
                            for j in range(4):
                                nc.gpsimd.dma_start(
                                    out=out_d[pq, j, :, nsl],
                                    in_=ob[32 * j : 32 * j + 8, nsl],
                                )
                if prev is not None and prev[0] != NQ - 1:
                    pq = prev[0]
                    for j in range(4):
                        nc.gpsimd.dma_start(
                            out=out_d[pq, j], in_=ob[32 * j : 32 * j + 8, :]
                        )
                prev = (qq, w1t, hq) if qq < NQ else None

    nc.compile()
    return nc


def _get_nc():
    if "nc" not in _BUILT:
        _BUILT["nc"] = _build()
    return _BUILT["nc"]


def _bf16_split(a):
    """Return (hi, lo) bf16 arrays with hi + lo ~= a (fp32)."""
    hi = a.astype(BF16)
    lo = (a - hi.astype(np.float32)).astype(BF16)
    return hi, lo


def _pack_weights(W0, b0, W1, b1):
    W0aug = np.empty((3, C * H), np.float32)
    W0aug[0] = W0[:, :, 0].reshape(-1)
    W0aug[1] = W0[:, :, 1].reshape(-1)
    W0aug[2] = b0.reshape(-1)
    if MM1_MODE == "bf16x2":
        Whi, Wlo = _bf16_split(W0aug)
        w0p = np.zeros((NQ, 128, 128), BF16)
        for q in range(NQ):
            for j in range(4):
                m = 4 * q + j
                sl = slice(128 * m, 128 * (m + 1))
                r = 32 * j
                w0p[q, r : r + 3, :] = Whi[:, sl]
                w0p[q, r + 3 : r + 6, :] = Whi[:, sl]
                w0p[q, r + 6 : r + 9, :] = Wlo[:, sl]
    else:
        w0p = np.zeros((NQ, 128, 128), np.float32)
        for q in range(NQ):
            for j in range(4):
                m = 4 * q + j
                w0p[q, 32 * j : 32 * j + 3, :] = W0aug[:, 128 * m : 128 * (m + 1)]

    w1p = np.zeros((NQ, 128, 128), np.float32)
    b1p = np.zeros((128, NQ), np.float32)
    for q in range(NQ):
        for j in range(4):
            for cl in range(4):
                ch = 16 * q + 4 * j + cl
                for o in range(OUT_DIM):
                    col = 32 * j + 2 * cl + o
                    w1p[q, 32 * cl : 32 * cl + 32, col] = W1[ch, o, :]
                    b1p[col, q] = b1[ch, o]
    return w0p, w1p, b1p


def _run(inputs, trace=False, trace_kwargs=None):
    from concourse.bass_utils import run_bass_kernel_spmd

    x = np.ascontiguousarray(np.asarray(inputs["x"], dtype=np.float32))
    W0 = np.asarray(inputs["W0"], dtype=np.float32)
    b0 = np.asarray(inputs["b0"], dtype=np.float32)
    W1 = np.asarray(inputs["W1"], dtype=np.float32)
    b1 = np.asarray(inputs["b1"], dtype=np.float32)

    w0p, w1p, b1p = _pack_weights(W0, b0, W1, b1)

    in_maps = []
    for k in range(NCORES):
        xs = x[k * BC : (k + 1) * BC]
        xa = np.zeros((3, BC), np.float32)
        xa[0] = xs[:, 0]
        xa[1] = xs[:, 1]
        xa[2] = 1.0
        if MM1_MODE == "bf16x2":
            hi, lo = _bf16_split(xa)
            xab = np.zeros((9, BC), BF16)
            xab[0:3] = hi  # pairs with Whi
            xab[3:5] = lo[0:2]  # pairs with Whi (lo of ones-row is 0)
            xab[6:9] = hi  # pairs with Wlo
        else:
            xab = xa
        in_maps.append({"xt": xab, "w0p": w0p, "w1p": w1p, "b1p": b1p})

    nc = _get_nc()
    kwargs = {}
    if trace:
        kwargs["trace"] = True
        kwargs.update(trace_kwargs or {})
    res = run_bass_kernel_spmd(nc, in_maps, core_ids=list(range(NCORES)), **kwargs)

    outs = []
    for k in range(NCORES):
        blk = res.results[k]["out"]  # [NQ, 4, 8, BC]
        blk = blk.reshape(NQ, 4, 4, OUT_DIM, BC)
        blk = np.transpose(blk, (4, 0, 1, 2, 3)).reshape(BC, C, OUT_DIM)
        outs.append(blk)
    full = np.concatenate(outs, axis=0).astype(np.float32, copy=False)
    return full, res


def kernel(**inputs) -> np.ndarray:
    out, _ = _run(inputs)
    return out


if __name__ == "__main__":
    rng = np.random.default_rng(0)
    demo = {
        "x": rng.standard_normal((B, IN_DIM), dtype=np.float32),
        "W0": rng.standard_normal((C, H, IN_DIM), dtype=np.float32),
        "b0": rng.standard_normal((C, H), dtype=np.float32),
        "W1": rng.standard_normal((C, OUT_DIM, H), dtype=np.float32),
        "b1": rng.standard_normal((C, OUT_DIM), dtype=np.float32),
    }
    out = kernel(**demo)
    print(out.shape, out.dtype)
